# revision 8
# baseline (speedup 1.0000x reference)
"""DeepSeek-V2-Lite MoE layer on 8 Trainium2 NeuronCores.

Strategy (expert-parallel, per the sharding hint):
  - Host computes the gate (256x64 matmul + softmax + top-6) in fp32 numpy --
    this is the token dispatch decision, which necessarily lives on the host
    since the host builds the per-core input shards ("all-to-all" realized as
    host-side gather/scatter under the full-IO contract).
  - Each core owns 8 routed experts (weights sharded on the expert axis) and
    a 1/8 slice of the shared expert intermediate dim (tensor-parallel).
  - Tokens routed to each expert are gathered host-side into a fixed-capacity
    [C] batch (C = max expert load rounded up; uniform so the single SPMD
    program is identical across cores).
  - Routed weights are quantized host-side to 1-byte fp8, halving the
    dominant weight DMA traffic: gate_up in e4m3 (x2048) so the gate_up
    matmuls run in fp8 DoubleRow mode (2 K-rows/cycle), down-proj in e3m4
    (x128, better mantissa) at bf16 rate. The token batch is sent as an
    e4m3 hi+lo pair (lo = exact residual) whose products accumulate in the
    same PSUM rows, so x quantization error is ~0.06% instead of 3%. All
    scales are powers of two, folded into the silu input scale and the
    host-prepared combine weights. The shared expert (which dominates the
    output norm) stays fully bf16.
  - All matmuls are token-stationary: the token batch is the stationary PE
    operand, the weights stream through as the moving operand.
  - The per-expert pipeline is software-pipelined (expert s's transpose +
    down-proj emitted after expert s+1's gate_up; down weights prefetched a
    stage ahead) and the shared-expert groups are slotted mid-stream where
    the PE has slack, so the kernel end is just the last expert's short
    down-proj chain.
  - Device applies the per-token routing weight; host combine is a pure
    gather-sum plus the 8-way shared-expert partial sum.
"""

import os
import numpy as np
import ml_dtypes

BF16 = ml_dtypes.bfloat16
E3M4 = ml_dtypes.float8_e3m4
E4M3 = ml_dtypes.float8_e4m3   # TRN flavor: max normal 240

HIDDEN = 2048
FFN = 1408
N_EXPERTS = 64
TOP_K = 6
SHARED_FFN = 2816          # 2 shared experts * FFN
T = 256
N_CORES = 8
EPC = N_EXPERTS // N_CORES  # experts per core = 8
SFS = SHARED_FFN // N_CORES  # shared-FFN slice per core = 352

SX = 32.0                   # x hi/lo e4m3 scale
SWG = 2048.0                # gate_up e4m3 weight scale
SWD = 128.0                 # down e3m4 weight scale

# gate/up column interleave: stream order [g0|u0|g1|u1|g2|u2], pair widths
PAIR_W = [512, 512, 384]
PAIR_OFF = [0, 1024, 2048]          # start col of each pair block (2*w wide)
N_KH = HIDDEN // 128                # 16 K-chunks over hidden
N_KP = N_KH // 2                    # 8 K-pairs (DoubleRow)
N_KF = FFN // 128                   # 11 K-chunks over FFN

_PROGRAM_CACHE = {}
LAST_RESULTS = None


def _route(x, gate_w):
    """fp32 softmax top-k routing, matching jax.lax.top_k tie-breaking
    (stable sort -> lowest index wins ties)."""
    logits = x @ gate_w.T                      # [T, E] fp32
    m = logits.max(axis=-1, keepdims=True)
    e = np.exp(logits - m)
    scores = e / e.sum(axis=-1, keepdims=True)
    ids = np.argsort(-scores, axis=-1, kind="stable")[:, :TOP_K]
    w = np.take_along_axis(scores, ids, axis=-1)
    w = w / (w.sum(axis=-1, keepdims=True) + 1e-20)
    return ids, w.astype(np.float32)


def _build_program(C):
    import concourse.bass as bass
    import concourse.bacc as bacc
    import concourse.mybir as mybir
    import concourse.tile as tile
    from concourse.masks import make_identity
    from contextlib import ExitStack

    f32 = mybir.dt.float32
    bf16 = mybir.dt.bfloat16
    f8e3 = mybir.dt.float8e3
    f8e4 = mybir.dt.float8e4
    SILU = mybir.ActivationFunctionType.Silu
    DR = mybir.MatmulPerfMode.DoubleRow
    C2 = 2 * C

    # Bacc (not plain Bass): its compile pipeline splits multi-wait
    # instructions into the 1-wait-per-instruction form TRN2 requires.
    nc = bacc.Bacc(None)

    # DRAM layouts are host-prepped into final SBUF layouts so every weight
    # DMA is contiguous per partition row.
    W_GU = N_KH * 2816
    W_DN = 2 * N_KF * 1024
    d_xt = nc.dram_tensor("xt", [EPC, 128, N_KH * C2], f8e4, kind="ExternalInput")
    d_wgu = nc.dram_tensor("wgu", [EPC, 128, W_GU], f8e4, kind="ExternalInput")
    d_wdn = nc.dram_tensor("wdn", [EPC, 128, W_DN], f8e3, kind="ExternalInput")
    d_wv = nc.dram_tensor("wv", [C, EPC], f32, kind="ExternalInput")
    d_xsh = nc.dram_tensor("xsh", [128, N_KH * 256], bf16, kind="ExternalInput")
    d_wsgu = nc.dram_tensor("wsgu", [128, N_KH * 2 * SFS], bf16, kind="ExternalInput")
    d_wsd = nc.dram_tensor("wsd", [128, 3 * 2048], bf16, kind="ExternalInput")
    d_yrt = nc.dram_tensor("yrt", [EPC, C, HIDDEN], bf16, kind="ExternalOutput")
    d_ysh = nc.dram_tensor("ysh", [T, HIDDEN], bf16, kind="ExternalOutput")

    with tile.TileContext(nc) as tc, ExitStack() as ctx:
        p_const = ctx.enter_context(tc.tile_pool(name="const", bufs=1))
        p_wgu = ctx.enter_context(tc.tile_pool(name="wgu", bufs=2))
        p_wdn = ctx.enter_context(tc.tile_pool(name="wdn", bufs=4))
        p_xt = ctx.enter_context(tc.tile_pool(name="xt", bufs=2))
        p_act = ctx.enter_context(tc.tile_pool(name="act", bufs=2))
        p_gs = ctx.enter_context(tc.tile_pool(name="gs", bufs=2))
        p_actT = ctx.enter_context(tc.tile_pool(name="actT", bufs=2))
        p_out = ctx.enter_context(tc.tile_pool(name="out", bufs=2))
        p_shg = ctx.enter_context(tc.tile_pool(name="shg", bufs=1))
        p_shd = ctx.enter_context(tc.tile_pool(name="shd", bufs=1))
        p_shx = ctx.enter_context(tc.tile_pool(name="shx", bufs=1))
        ps_gu = ctx.enter_context(tc.tile_pool(name="ps_gu", bufs=2, space="PSUM"))
        ps_y = ctx.enter_context(tc.tile_pool(name="ps_y", bufs=2, space="PSUM"))
        ps_t = ctx.enter_context(tc.tile_pool(name="ps_t", bufs=2, space="PSUM"))

        ident = p_const.tile([128, 128], bf16)
        make_identity(nc, ident[:])
        wv_t = p_const.tile([C, EPC], f32)
        nc.sync.dma_start(out=wv_t[:], in_=d_wv[:])

        def stage_a(s):
            """gate+up projection for expert s -> act tile (bf16, x SX*SWG).

            DoubleRow fp8: the stationary token batch is [hi | lo] e4m3
            columns; lo (the exact quantization residual of hi) accumulates
            into the same PSUM rows, cancelling the x quantization error.
            """
            xt = p_xt.tile([128, N_KH, C2], f8e4, tag="xt")
            nc.sync.dma_start(out=xt[:], in_=d_xt[s])
            act = p_act.tile([C, FFN], bf16, tag="act")
            # ---- gate+up, pair-major over 3 (g,u) column pairs ----
            for pr in range(3):
                w = PAIR_W[pr]
                wg = p_wgu.tile([128, N_KP, 2, 2 * w], f8e4, tag="wgu")
                nc.sync.dma_start(
                    out=wg[:],
                    in_=d_wgu[s, :, N_KH * PAIR_OFF[pr]:
                              N_KH * (PAIR_OFF[pr] + 2 * w)],
                )
                pg = ps_gu.tile([C, 1024], mybir.dt.float32, tag="pg")
                for p in range(N_KP):
                    lhs_hi = xt[:, 2 * p:2 * p + 2, 0:C]
                    lhs_lo = xt[:, 2 * p:2 * p + 2, C:C2]
                    for ci in range(2):  # g cols, u cols
                        rhs = wg[:, p, :, ci * w:(ci + 1) * w]
                        out_sl = pg[:, ci * 512: ci * 512 + w]
                        nc.tensor.matmul(
                            out_sl, lhs_hi, rhs,
                            start=(p == 0), stop=False, perf_mode=DR,
                        )
                        nc.tensor.matmul(
                            out_sl, lhs_lo, rhs,
                            start=False, stop=(p == N_KP - 1), perf_mode=DR,
                        )
                # psum g is x SX*SWG; true silu needs the descale on input.
                gs = p_gs.tile([C, 512], mybir.dt.float32, tag="gs")
                nc.scalar.activation(gs[:, :w], pg[:, :w], SILU, scale=1.0 / (SX * SWG))
                # act carries x SX*SWG from the u psum; folded into wv at drain.
                nc.vector.tensor_mul(
                    act[:, pr * 512: pr * 512 + w], gs[:, :w], pg[:, 512:512 + w]
                )
            return act

        def prefetch_d(s):
            """issue down-proj weight DMAs for expert s ahead of stage_b."""
            wds = []
            for h in range(2):
                wd = p_wdn.tile([128, N_KF * 1024], f8e3, tag="wdn")
                nc.sync.dma_start(
                    out=wd[:],
                    in_=d_wdn[s, :, h * N_KF * 1024:(h + 1) * N_KF * 1024],
                )
                wds.append(wd)
            return wds

        def stage_b(s, act, wds):
            """transpose + down projection + weighted drain for expert s."""
            actT = p_actT.tile([128, N_KF * C], bf16, tag="actT")
            for j in range(N_KF):
                pt = ps_t.tile([128, 128], bf16, tag="pt")
                nc.tensor.transpose(
                    pt[:, :C], act[:, j * 128:(j + 1) * 128], ident[:C, :C]
                )
                nc.vector.tensor_copy(actT[:, j * C:(j + 1) * C], pt[:, :C])
            out_sb = p_out.tile([C, HIDDEN], bf16, tag="out")
            for h in range(2):
                wd = wds[h]
                for n in range(2):
                    py = ps_y.tile([C, 512], mybir.dt.float32, tag="py")
                    for k in range(N_KF):
                        nc.tensor.matmul(
                            py[:],
                            actT[:, k * C:(k + 1) * C],
                            wd[:, k * 1024 + n * 512: k * 1024 + (n + 1) * 512],
                            start=(k == 0), stop=(k == N_KF - 1),
                        )
                    # routed combine weight (with the 1/(SX*SWG*SWD) descale
                    # baked in on the host) folded in during PSUM drain
                    nc.vector.tensor_scalar_mul(
                        out_sb[:, h * 1024 + n * 512: h * 1024 + (n + 1) * 512],
                        py[:], wv_t[:, s:s + 1]
                    )
            nc.sync.dma_start(out=d_yrt[s], in_=out_sb[:])

        def shared_dma():
            xsh = p_shx.tile([128, N_KH * 256], bf16, tag="xsh")
            nc.sync.dma_start(out=xsh[:], in_=d_xsh[:])
            wsg = p_shg.tile([128, N_KH * 2 * SFS], bf16, tag="wsg")
            nc.sync.dma_start(out=wsg[:], in_=d_wsgu[:])
            wsd = p_shd.tile([128, 3 * 2048], bf16, tag="wsd")
            nc.sync.dma_start(out=wsd[:], in_=d_wsd[:])
            return xsh, wsg, wsd

        def shared_group(g, sh):
            xsh, wsg, wsd = sh
            pg = ps_gu.tile([128, 1024], mybir.dt.float32, tag="pg")
            for k in range(N_KH):
                lhs = xsh[:, k * 256 + g * 128: k * 256 + g * 128 + 128]
                nc.tensor.matmul(
                    pg[:, 0:SFS], lhs, wsg[:, k * 2 * SFS: k * 2 * SFS + SFS],
                    start=(k == 0), stop=(k == N_KH - 1),
                )
                nc.tensor.matmul(
                    pg[:, 512:512 + SFS],
                    lhs, wsg[:, k * 2 * SFS + SFS: (k + 1) * 2 * SFS],
                    start=(k == 0), stop=(k == N_KH - 1),
                )
            gs = p_gs.tile([128, 512], mybir.dt.float32, tag="gs")
            nc.scalar.activation(gs[:, :SFS], pg[:, :SFS], SILU)
            act_sh = p_act.tile([128, SFS], bf16, tag="act")
            nc.vector.tensor_mul(act_sh[:], gs[:, :SFS], pg[:, 512:512 + SFS])
            actT_sh = p_actT.tile([128, 3 * 128], bf16, tag="actT")
            # rows 96:128 of the last K-chunk pair with zero weight rows;
            # zero them so junk*0 can't produce NaN
            nc.vector.memset(actT_sh[:], 0.0)
            for j, wj in enumerate([128, 128, 96]):
                pt = ps_t.tile([128, 128], bf16, tag="pt")
                nc.tensor.transpose(
                    pt[:wj, :], act_sh[:, j * 128: j * 128 + wj], ident[:, :]
                )
                nc.vector.tensor_copy(
                    actT_sh[:wj, j * 128:(j + 1) * 128], pt[:wj, :]
                )
            out_sh = p_out.tile([128, HIDDEN], bf16, tag="out")
            for h in range(2):
                for n in range(2):
                    py = ps_y.tile([128, 512], mybir.dt.float32, tag="py")
                    for k in range(3):
                        nc.tensor.matmul(
                            py[:],
                            actT_sh[:, k * 128:(k + 1) * 128],
                            wsd[:, k * 2048 + h * 1024 + n * 512:
                                k * 2048 + h * 1024 + (n + 1) * 512],
                            start=(k == 0), stop=(k == 2),
                        )
                    nc.vector.tensor_copy(
                        out_sh[:, h * 1024 + n * 512: h * 1024 + (n + 1) * 512],
                        py[:],
                    )
            nc.sync.dma_start(out=d_ysh[g * 128:(g + 1) * 128, :], in_=out_sh[:])

        # Software pipeline: expert s's transpose+down-proj is emitted after
        # expert s+1's gate_up matmuls, so the PE never stalls waiting for the
        # scalar/vector act computation; down weights are prefetched a full
        # gate_up stage ahead. Shared-expert groups fill PE slack mid-stream
        # so the program ends on the last expert's short down-proj chain.
        sh = None
        acts = {}
        wds = {}
        acts[0] = stage_a(0)
        for s in range(1, EPC):
            wds[s - 1] = prefetch_d(s - 1)
            if s == 4:
                sh = shared_dma()
            acts[s] = stage_a(s)
            stage_b(s - 1, acts.pop(s - 1), wds.pop(s - 1))
            if s == 5:
                shared_group(0, sh)
            elif s == 6:
                shared_group(1, sh)
        wds[EPC - 1] = prefetch_d(EPC - 1)
        stage_b(EPC - 1, acts.pop(EPC - 1), wds.pop(EPC - 1))

    if not nc.is_finalized():
        nc.finalize()
    return nc


def _sbufize(a, kdim):
    """[K*128, N] -> [128, K*N] SBUF layout (K-chunks along free dim)."""
    K = a.shape[0] // 128
    return np.ascontiguousarray(
        a.reshape(K, 128, -1).transpose(1, 0, 2).reshape(128, -1)
    )


def _prepare(hidden_states, gate_w, w_gate_up, w_down, ws_gate_up, ws_down):
    x = np.asarray(hidden_states, dtype=np.float32).reshape(T, HIDDEN)
    gate_w = np.asarray(gate_w, dtype=np.float32)

    ids, tw = _route(x, gate_w)

    # per-expert token lists + positions
    lists = [[] for _ in range(N_EXPERTS)]
    pos = np.zeros((T, TOP_K), dtype=np.int64)
    for t in range(T):
        for i in range(TOP_K):
            e = ids[t, i]
            pos[t, i] = len(lists[e])
            lists[e].append(t)
    maxload = max(len(l) for l in lists)
    C = max(32, -(-maxload // 16) * 16)
    assert C <= 128, f"expert overload {maxload}: splitting not implemented"

    xT = np.ascontiguousarray(x.T)  # fp32 [H, T]

    # column permutation interleaving gate/up rows into [g0|u0|g1|u1|g2|u2]
    perm = np.concatenate([
        np.concatenate([np.arange(o, o + w), FFN + np.arange(o, o + w)])
        for o, w in zip([0, 512, 1024], PAIR_W)
    ])

    w_gate_up = np.asarray(w_gate_up)
    w_down = np.asarray(w_down)
    ws_gate_up = np.asarray(ws_gate_up)
    ws_down = np.asarray(ws_down)

    def q_e4(a):
        return np.clip(a * SWG, -240.0, 240.0).astype(E4M3)

    def q_e3(a):
        return np.clip(a * SWD, -15.5, 15.5).astype(E3M4)

    in_maps = []
    for c in range(N_CORES):
        # routed expert weights, token batches
        wgu = np.empty((EPC, 128, N_KH * 2816), dtype=E4M3)
        wdn = np.empty((EPC, 128, 2 * N_KF * 1024), dtype=E3M4)
        xts = np.zeros((EPC, 128, N_KH * 2 * C), dtype=E4M3)
        wv = np.zeros((C, EPC), dtype=np.float32)
        for s in range(EPC):
            e = c * EPC + s
            wt = q_e4(w_gate_up[e][perm].T)             # [H, 2816] interleaved
            off = 0
            for o, w in zip(PAIR_OFF, PAIR_W):
                blk = _sbufize(wt[:, o:o + 2 * w], N_KH)  # [128, 16*2w]
                wgu[s, :, off:off + blk.shape[1]] = blk
                off += blk.shape[1]
            wdT = q_e3(w_down[e].T)                      # [F, H]
            for h in range(2):
                wdn[s, :, h * N_KF * 1024:(h + 1) * N_KF * 1024] = _sbufize(
                    wdT[:, h * 1024:(h + 1) * 1024], N_KF
                )
            toks = lists[e]
            n = len(toks)
            if n:
                xe = np.zeros((HIDDEN, 2 * C), dtype=np.float32)
                xs = xT[:, toks] * SX
                hi = np.clip(xs, -240.0, 240.0).astype(E4M3)
                lo = (xs - hi.astype(np.float32)).astype(E4M3)
                xe[:, :n] = hi
                xe[:, C:C + n] = lo
                xts[s] = _sbufize(xe, N_KH).astype(E4M3)
                # per-token routing weights in expert order, with the fp8
                # scales (SX*SWG on the gate_up path x SWD on down) divided out
                wcol = np.zeros(C, dtype=np.float32)
                for i in range(TOP_K):
                    sel = ids[:, i] == e
                    wcol[pos[sel, i]] = tw[sel, i]
                wv[:, s] = wcol / (SX * SWG * SWD)
        # shared expert slice (tensor-parallel on intermediate dim)
        g_sl = ws_gate_up[c * SFS:(c + 1) * SFS]            # [352, H]
        u_sl = ws_gate_up[SHARED_FFN + c * SFS: SHARED_FFN + (c + 1) * SFS]
        wsgu = _sbufize(
            np.concatenate([g_sl, u_sl], axis=0).T.astype(BF16), N_KH
        )  # [128, 16*704]
        wsdT = ws_down[:, c * SFS:(c + 1) * SFS].T.astype(BF16)  # [352, H]
        wsd_pad = np.zeros((384, HIDDEN), dtype=BF16)
        wsd_pad[:SFS] = wsdT
        wsd = _sbufize(wsd_pad, 3)                          # [128, 3*2048]
        xsh = _sbufize(xT.astype(BF16), N_KH)               # [128, 16*256]
        in_maps.append({
            "xt": xts, "wgu": wgu, "wdn": wdn, "wv": wv,
            "xsh": xsh, "wsgu": wsgu, "wsd": wsd,
        })
    return C, ids, pos, in_maps


def _combine(C, ids, pos, results):
    # ---- combine: gather-sum of weighted routed rows + shared partials ----
    y_all = np.stack([r["yrt"].astype(np.float32) for r in results])  # [8, EPC, C, H]
    y_flat = y_all.reshape(N_EXPERTS * C, HIDDEN)
    G = ids * C + pos                                       # [T, 6]
    routed = y_flat[G].sum(axis=1)
    shared = np.sum([r["ysh"].astype(np.float32) for r in results], axis=0)
    out = routed + shared
    return out.reshape(1, T, HIDDEN).astype(np.float32)


def kernel(hidden_states, gate_w, w_gate_up, w_down, ws_gate_up, ws_down):
    global LAST_RESULTS
    C, ids, pos, in_maps = _prepare(
        hidden_states, gate_w, w_gate_up, w_down, ws_gate_up, ws_down
    )

    if C not in _PROGRAM_CACHE:
        _PROGRAM_CACHE[C] = _build_program(C)
    nc = _PROGRAM_CACHE[C]

    from concourse.bass_utils import run_bass_kernel_spmd
    res = run_bass_kernel_spmd(
        nc, in_maps, list(range(N_CORES)),
        trace=bool(os.environ.get("MOE_KERNEL_TRACE")),
    )
    LAST_RESULTS = res
    return _combine(C, ids, pos, res.results)


# revision 15
# speedup vs baseline: 1.0630x; 1.0630x over previous
"""DeepSeek-V2-Lite MoE layer on 8 Trainium2 NeuronCores.

Strategy (expert-parallel, per the sharding hint):
  - Host computes the gate (256x64 matmul + softmax + top-6) in fp32 numpy --
    this is the token dispatch decision, which necessarily lives on the host
    since the host builds the per-core input shards ("all-to-all" realized as
    host-side gather/scatter under the full-IO contract).
  - Each core owns 8 routed experts (weights sharded on the expert axis) and
    a 1/8 slice of the shared expert intermediate dim (tensor-parallel).
  - Tokens routed to each expert are gathered host-side into a fixed-capacity
    [C] batch (C = max expert load rounded up; uniform so the single SPMD
    program is identical across cores).
  - Routed weights are quantized host-side to 1-byte fp8, halving the
    dominant weight DMA traffic: gate_up in e4m3 (x2048) so the gate_up
    matmuls run in fp8 DoubleRow mode (2 K-rows/cycle), down-proj in e3m4
    (x128, better mantissa) at bf16 rate. The token batch is sent as an
    e4m3 hi+lo pair (lo = exact residual) whose products accumulate in the
    same PSUM rows, so x quantization error is ~0.06% instead of 3%. All
    scales are powers of two, folded into the silu input scale and the
    host-prepared combine weights. The shared expert (which dominates the
    output norm) stays fully bf16.
  - All matmuls are token-stationary: the token batch is the stationary PE
    operand, the weights stream through as the moving operand.
  - The per-expert pipeline is software-pipelined (expert s's transpose +
    down-proj emitted after expert s+1's gate_up; down weights prefetched a
    stage ahead) and the shared-expert groups are slotted mid-stream where
    the PE has slack, so the kernel end is just the last expert's short
    down-proj chain.
  - Device applies the per-token routing weight; host combine is a pure
    gather-sum plus the 8-way shared-expert partial sum.
"""

import os
import numpy as np
import ml_dtypes

BF16 = ml_dtypes.bfloat16
E3M4 = ml_dtypes.float8_e3m4
E4M3 = ml_dtypes.float8_e4m3   # TRN flavor: max normal 240

HIDDEN = 2048
FFN = 1408
N_EXPERTS = 64
TOP_K = 6
SHARED_FFN = 2816          # 2 shared experts * FFN
T = 256
N_CORES = 8
EPC = N_EXPERTS // N_CORES  # experts per core = 8
SFS = SHARED_FFN // N_CORES  # shared-FFN slice per core = 352

SX = 32.0                   # x hi/lo e4m3 scale
SWG = 2048.0                # gate_up e4m3 weight scale
SWD = 128.0                 # down e3m4 weight scale

# gate/up column interleave: stream order [g0|u0|g1|u1|g2|u2], pair widths
PAIR_W = [512, 512, 384]
PAIR_OFF = [0, 1024, 2048]          # start col of each pair block (2*w wide)
N_KH = HIDDEN // 128                # 16 K-chunks over hidden
N_KP = N_KH // 2                    # 8 K-pairs (DoubleRow)
N_KF = FFN // 128                   # 11 K-chunks over FFN

_PROGRAM_CACHE = {}
LAST_RESULTS = None


def _route(x, gate_w):
    """fp32 softmax top-k routing, matching jax.lax.top_k tie-breaking
    (stable sort -> lowest index wins ties)."""
    logits = x @ gate_w.T                      # [T, E] fp32
    m = logits.max(axis=-1, keepdims=True)
    e = np.exp(logits - m)
    scores = e / e.sum(axis=-1, keepdims=True)
    ids = np.argsort(-scores, axis=-1, kind="stable")[:, :TOP_K]
    w = np.take_along_axis(scores, ids, axis=-1)
    w = w / (w.sum(axis=-1, keepdims=True) + 1e-20)
    return ids, w.astype(np.float32)


def _build_program(C):
    import concourse.bass as bass
    import concourse.bacc as bacc
    import concourse.mybir as mybir
    import concourse.tile as tile
    from concourse.masks import make_identity
    from contextlib import ExitStack

    f32 = mybir.dt.float32
    bf16 = mybir.dt.bfloat16
    f8e3 = mybir.dt.float8e3
    f8e4 = mybir.dt.float8e4
    SILU = mybir.ActivationFunctionType.Silu
    DR = mybir.MatmulPerfMode.DoubleRow
    C2 = 2 * C

    # Bacc (not plain Bass): its compile pipeline splits multi-wait
    # instructions into the 1-wait-per-instruction form TRN2 requires.
    nc = bacc.Bacc(None)

    # DRAM layouts are host-prepped into final SBUF layouts so every weight
    # DMA is contiguous per partition row.
    W_GU = N_KH * 2816
    W_DN = 2 * N_KF * 1024
    d_xt = nc.dram_tensor("xt", [EPC, 128, N_KH * C2], f8e4, kind="ExternalInput")
    d_wgu = nc.dram_tensor("wgu", [EPC, 128, W_GU], f8e4, kind="ExternalInput")
    d_wdn = nc.dram_tensor("wdn", [EPC, 128, W_DN], f8e3, kind="ExternalInput")
    d_wv = nc.dram_tensor("wv", [C, EPC], f32, kind="ExternalInput")
    # [I_C; I_C] stacked: transposes a [C2, 128] hi|lo block while summing the
    # hi and lo halves into one [128, C] result.
    d_mm = nc.dram_tensor("mm", [C2, C], bf16, kind="ExternalInput")
    d_xsh = nc.dram_tensor("xsh", [128, N_KH * 256], bf16, kind="ExternalInput")
    d_wsgu = nc.dram_tensor("wsgu", [128, N_KH * 2 * SFS], bf16, kind="ExternalInput")
    d_wsd = nc.dram_tensor("wsd", [128, 3 * 2048], bf16, kind="ExternalInput")
    d_yrt = nc.dram_tensor("yrt", [EPC, C, HIDDEN], bf16, kind="ExternalOutput")
    d_ysh = nc.dram_tensor("ysh", [T, HIDDEN], bf16, kind="ExternalOutput")

    with tile.TileContext(nc) as tc, ExitStack() as ctx:
        p_const = ctx.enter_context(tc.tile_pool(name="const", bufs=1))
        p_wgu = ctx.enter_context(tc.tile_pool(name="wgu", bufs=2))
        p_wdn = ctx.enter_context(tc.tile_pool(name="wdn", bufs=4))
        p_xt = ctx.enter_context(tc.tile_pool(name="xt", bufs=2))
        p_act = ctx.enter_context(tc.tile_pool(name="act", bufs=2))
        p_gs = ctx.enter_context(tc.tile_pool(name="gs", bufs=2))
        p_actT = ctx.enter_context(tc.tile_pool(name="actT", bufs=2))
        p_out = ctx.enter_context(tc.tile_pool(name="out", bufs=2))
        p_shg = ctx.enter_context(tc.tile_pool(name="shg", bufs=1))
        p_shd = ctx.enter_context(tc.tile_pool(name="shd", bufs=1))
        p_shx = ctx.enter_context(tc.tile_pool(name="shx", bufs=1))
        ps_gu = ctx.enter_context(tc.tile_pool(name="ps_gu", bufs=2, space="PSUM"))
        ps_y = ctx.enter_context(tc.tile_pool(name="ps_y", bufs=2, space="PSUM"))
        ps_t = ctx.enter_context(tc.tile_pool(name="ps_t", bufs=2, space="PSUM"))

        ident = p_const.tile([128, 128], bf16)
        make_identity(nc, ident[:])
        wv_t = p_const.tile([C, EPC], f32)
        nc.sync.dma_start(out=wv_t[:], in_=d_wv[:])
        mm_t = p_const.tile([C2, C], bf16)
        nc.sync.dma_start(out=mm_t[:], in_=d_mm[:])

        def stage_a(s):
            """gate+up projection for expert s -> gu tile (bf16, x SX*SWG).

            DoubleRow fp8 at full rate: the stationary token batch carries the
            e4m3 hi and lo (exact residual) copies as separate columns
            (M = 2C <= 128, free on the PE), so each weight column streams
            through once at 2 K-rows/cycle. The hi+lo merge happens later,
            fused into the transpose matmul. Result rows: [hi(C) | lo(C)].
            """
            xt = p_xt.tile([128, N_KH, C2], f8e4, tag="xt")
            nc.sync.dma_start(out=xt[:], in_=d_xt[s])
            gu = p_act.tile([C2, 2 * FFN], bf16, tag="act")  # [g(1408) | u(1408)]
            # ---- gate+up, pair-major over 3 (g,u) column pairs ----
            for pr in range(3):
                w = PAIR_W[pr]
                o = PAIR_OFF[pr] // 2  # global g/u col offset of this block
                wg = p_wgu.tile([128, N_KP, 2, 2 * w], f8e4, tag="wgu")
                nc.sync.dma_start(
                    out=wg[:],
                    in_=d_wgu[s, :, N_KH * PAIR_OFF[pr]:
                              N_KH * (PAIR_OFF[pr] + 2 * w)],
                )
                pg = ps_gu.tile([C2, 1024], mybir.dt.float32, tag="pg")
                if w == 512:
                    # fp8 DoubleRow, 2 K-rows/cycle (hw requires 512-wide
                    # moving slices -- 384-wide DR returns garbage)
                    for p in range(N_KP):
                        lhs = xt[:, 2 * p:2 * p + 2, :]
                        for ci in range(2):  # g cols, u cols
                            nc.tensor.matmul(
                                pg[:, ci * 512: ci * 512 + w],
                                lhs, wg[:, p, :, ci * w:(ci + 1) * w],
                                start=(p == 0), stop=(p == N_KP - 1), perf_mode=DR,
                            )
                else:
                    # 384-wide tail block: plain single-rate fp8 matmuls
                    for k in range(N_KH):
                        lhs = xt[:, k, :]
                        for ci in range(2):
                            nc.tensor.matmul(
                                pg[:, ci * 512: ci * 512 + w],
                                lhs, wg[:, k // 2, k % 2, ci * w:(ci + 1) * w],
                                start=(k == 0), stop=(k == N_KH - 1),
                            )
                # drain psum (x SX*SWG) to bf16; hi/lo rows merge in stage_b
                nc.vector.tensor_copy(gu[:, o:o + w], pg[:, 0:w])
                nc.vector.tensor_copy(
                    gu[:, FFN + o:FFN + o + w], pg[:, 512:512 + w]
                )
            return gu

        def prefetch_d(s):
            """issue down-proj weight DMAs for expert s ahead of stage_b."""
            wds = []
            for h in range(2):
                wd = p_wdn.tile([128, N_KF * 1024], f8e3, tag="wdn")
                nc.sync.dma_start(
                    out=wd[:],
                    in_=d_wdn[s, :, h * N_KF * 1024:(h + 1) * N_KF * 1024],
                )
                wds.append(wd)
            return wds

        def stage_b(s, gu, wds):
            """merge-transpose + silu*u + down projection for expert s.

            Each [C2, 128] hi|lo block of g (and u) is transposed by the
            [I_C; I_C] matrix, which also sums the hi and lo halves -- one
            matmul per 128-chunk yields the merged, transposed fp32 result.
            """
            actT = p_actT.tile([128, N_KF * C], bf16, tag="actT")
            for j in range(N_KF):
                ptg = ps_t.tile([128, C], mybir.dt.float32, tag="pt")
                nc.tensor.matmul(
                    ptg[:], gu[:, j * 128:(j + 1) * 128], mm_t[:],
                    start=True, stop=True,
                )
                ptu = ps_t.tile([128, C], mybir.dt.float32, tag="pt")
                nc.tensor.matmul(
                    ptu[:], gu[:, FFN + j * 128:FFN + (j + 1) * 128], mm_t[:],
                    start=True, stop=True,
                )
                # true silu needs the SX*SWG descale on input; actT keeps the
                # x SX*SWG factor from u (folded into wv at the down drain).
                gst = p_gs.tile([128, C], mybir.dt.float32, tag="gs")
                nc.scalar.activation(gst[:], ptg[:], SILU, scale=1.0 / (SX * SWG))
                nc.vector.tensor_mul(actT[:, j * C:(j + 1) * C], gst[:], ptu[:])
            out_sb = p_out.tile([C, HIDDEN], bf16, tag="out")
            for h in range(2):
                wd = wds[h]
                for n in range(2):
                    py = ps_y.tile([C, 512], mybir.dt.float32, tag="py")
                    for k in range(N_KF):
                        nc.tensor.matmul(
                            py[:],
                            actT[:, k * C:(k + 1) * C],
                            wd[:, k * 1024 + n * 512: k * 1024 + (n + 1) * 512],
                            start=(k == 0), stop=(k == N_KF - 1),
                        )
                    # routed combine weight (with the 1/(SX*SWG*SWD) descale
                    # baked in on the host) folded in during PSUM drain
                    nc.vector.tensor_scalar_mul(
                        out_sb[:, h * 1024 + n * 512: h * 1024 + (n + 1) * 512],
                        py[:], wv_t[:, s:s + 1]
                    )
            nc.sync.dma_start(out=d_yrt[s], in_=out_sb[:])

        def shared_dma():
            xsh = p_shx.tile([128, N_KH * 256], bf16, tag="xsh")
            nc.sync.dma_start(out=xsh[:], in_=d_xsh[:])
            wsg = p_shg.tile([128, N_KH * 2 * SFS], bf16, tag="wsg")
            nc.sync.dma_start(out=wsg[:], in_=d_wsgu[:])
            wsd = p_shd.tile([128, 3 * 2048], bf16, tag="wsd")
            nc.sync.dma_start(out=wsd[:], in_=d_wsd[:])
            return xsh, wsg, wsd

        def shared_group(g, sh):
            xsh, wsg, wsd = sh
            pg = ps_gu.tile([128, 1024], mybir.dt.float32, tag="pg")
            for k in range(N_KH):
                lhs = xsh[:, k * 256 + g * 128: k * 256 + g * 128 + 128]
                nc.tensor.matmul(
                    pg[:, 0:SFS], lhs, wsg[:, k * 2 * SFS: k * 2 * SFS + SFS],
                    start=(k == 0), stop=(k == N_KH - 1),
                )
                nc.tensor.matmul(
                    pg[:, 512:512 + SFS],
                    lhs, wsg[:, k * 2 * SFS + SFS: (k + 1) * 2 * SFS],
                    start=(k == 0), stop=(k == N_KH - 1),
                )
            gs = p_gs.tile([128, 512], mybir.dt.float32, tag="gs")
            nc.scalar.activation(gs[:, :SFS], pg[:, :SFS], SILU)
            act_sh = p_act.tile([128, SFS], bf16, tag="act")
            nc.vector.tensor_mul(act_sh[:], gs[:, :SFS], pg[:, 512:512 + SFS])
            actT_sh = p_actT.tile([128, 3 * 128], bf16, tag="actT")
            # rows 96:128 of the last K-chunk pair with zero weight rows;
            # zero them so junk*0 can't produce NaN
            nc.vector.memset(actT_sh[:], 0.0)
            for j, wj in enumerate([128, 128, 96]):
                pt = ps_t.tile([128, 128], bf16, tag="pt")
                nc.tensor.transpose(
                    pt[:wj, :], act_sh[:, j * 128: j * 128 + wj], ident[:, :]
                )
                nc.vector.tensor_copy(
                    actT_sh[:wj, j * 128:(j + 1) * 128], pt[:wj, :]
                )
            out_sh = p_out.tile([128, HIDDEN], bf16, tag="out")
            for h in range(2):
                for n in range(2):
                    py = ps_y.tile([128, 512], mybir.dt.float32, tag="py")
                    for k in range(3):
                        nc.tensor.matmul(
                            py[:],
                            actT_sh[:, k * 128:(k + 1) * 128],
                            wsd[:, k * 2048 + h * 1024 + n * 512:
                                k * 2048 + h * 1024 + (n + 1) * 512],
                            start=(k == 0), stop=(k == 2),
                        )
                    nc.vector.tensor_copy(
                        out_sh[:, h * 1024 + n * 512: h * 1024 + (n + 1) * 512],
                        py[:],
                    )
            nc.sync.dma_start(out=d_ysh[g * 128:(g + 1) * 128, :], in_=out_sh[:])

        # Software pipeline: expert s's transpose+down-proj is emitted after
        # expert s+1's gate_up matmuls, so the PE never stalls waiting for the
        # scalar/vector act computation; down weights are prefetched a full
        # gate_up stage ahead. Shared-expert groups fill PE slack mid-stream
        # so the program ends on the last expert's short down-proj chain.
        sh = None
        acts = {}
        wds = {}
        acts[0] = stage_a(0)
        for s in range(1, EPC):
            wds[s - 1] = prefetch_d(s - 1)
            if s == 4:
                sh = shared_dma()
            acts[s] = stage_a(s)
            stage_b(s - 1, acts.pop(s - 1), wds.pop(s - 1))
            if s == 5:
                shared_group(0, sh)
            elif s == 6:
                shared_group(1, sh)
        wds[EPC - 1] = prefetch_d(EPC - 1)
        stage_b(EPC - 1, acts.pop(EPC - 1), wds.pop(EPC - 1))

    if not nc.is_finalized():
        nc.finalize()
    return nc


def _sbufize(a, kdim):
    """[K*128, N] -> [128, K*N] SBUF layout (K-chunks along free dim)."""
    K = a.shape[0] // 128
    return np.ascontiguousarray(
        a.reshape(K, 128, -1).transpose(1, 0, 2).reshape(128, -1)
    )


def _prepare(hidden_states, gate_w, w_gate_up, w_down, ws_gate_up, ws_down):
    x = np.asarray(hidden_states, dtype=np.float32).reshape(T, HIDDEN)
    gate_w = np.asarray(gate_w, dtype=np.float32)

    ids, tw = _route(x, gate_w)

    # per-expert token lists + positions
    lists = [[] for _ in range(N_EXPERTS)]
    pos = np.zeros((T, TOP_K), dtype=np.int64)
    for t in range(T):
        for i in range(TOP_K):
            e = ids[t, i]
            pos[t, i] = len(lists[e])
            lists[e].append(t)
    maxload = max(len(l) for l in lists)
    C = max(32, -(-maxload // 16) * 16)
    assert C <= 128, f"expert overload {maxload}: splitting not implemented"

    xT = np.ascontiguousarray(x.T)  # fp32 [H, T]

    # column permutation interleaving gate/up rows into [g0|u0|g1|u1|g2|u2]
    perm = np.concatenate([
        np.concatenate([np.arange(o, o + w), FFN + np.arange(o, o + w)])
        for o, w in zip([0, 512, 1024], PAIR_W)
    ])

    w_gate_up = np.asarray(w_gate_up)
    w_down = np.asarray(w_down)
    ws_gate_up = np.asarray(ws_gate_up)
    ws_down = np.asarray(ws_down)

    def q_e4(a):
        return np.clip(a * SWG, -240.0, 240.0).astype(E4M3)

    def q_e3(a):
        return np.clip(a * SWD, -15.5, 15.5).astype(E3M4)

    in_maps = []
    for c in range(N_CORES):
        # routed expert weights, token batches
        wgu = np.empty((EPC, 128, N_KH * 2816), dtype=E4M3)
        wdn = np.empty((EPC, 128, 2 * N_KF * 1024), dtype=E3M4)
        xts = np.zeros((EPC, 128, N_KH * 2 * C), dtype=E4M3)
        wv = np.zeros((C, EPC), dtype=np.float32)
        for s in range(EPC):
            e = c * EPC + s
            wt = q_e4(w_gate_up[e][perm].T)             # [H, 2816] interleaved
            off = 0
            for o, w in zip(PAIR_OFF, PAIR_W):
                blk = _sbufize(wt[:, o:o + 2 * w], N_KH)  # [128, 16*2w]
                wgu[s, :, off:off + blk.shape[1]] = blk
                off += blk.shape[1]
            wdT = q_e3(w_down[e].T)                      # [F, H]
            for h in range(2):
                wdn[s, :, h * N_KF * 1024:(h + 1) * N_KF * 1024] = _sbufize(
                    wdT[:, h * 1024:(h + 1) * 1024], N_KF
                )
            toks = lists[e]
            n = len(toks)
            if n:
                xe = np.zeros((HIDDEN, 2 * C), dtype=np.float32)
                xs = xT[:, toks] * SX
                hi = np.clip(xs, -240.0, 240.0).astype(E4M3)
                lo = (xs - hi.astype(np.float32)).astype(E4M3)
                xe[:, :n] = hi
                xe[:, C:C + n] = lo
                xts[s] = _sbufize(xe, N_KH).astype(E4M3)
                # per-token routing weights in expert order, with the fp8
                # scales (SX*SWG on the gate_up path x SWD on down) divided out
                wcol = np.zeros(C, dtype=np.float32)
                for i in range(TOP_K):
                    sel = ids[:, i] == e
                    wcol[pos[sel, i]] = tw[sel, i]
                wv[:, s] = wcol / (SX * SWG * SWD)
        # shared expert slice (tensor-parallel on intermediate dim)
        g_sl = ws_gate_up[c * SFS:(c + 1) * SFS]            # [352, H]
        u_sl = ws_gate_up[SHARED_FFN + c * SFS: SHARED_FFN + (c + 1) * SFS]
        wsgu = _sbufize(
            np.concatenate([g_sl, u_sl], axis=0).T.astype(BF16), N_KH
        )  # [128, 16*704]
        wsdT = ws_down[:, c * SFS:(c + 1) * SFS].T.astype(BF16)  # [352, H]
        wsd_pad = np.zeros((384, HIDDEN), dtype=BF16)
        wsd_pad[:SFS] = wsdT
        wsd = _sbufize(wsd_pad, 3)                          # [128, 3*2048]
        xsh = _sbufize(xT.astype(BF16), N_KH)               # [128, 16*256]
        mm = np.vstack([np.eye(C), np.eye(C)]).astype(BF16)
        in_maps.append({
            "xt": xts, "wgu": wgu, "wdn": wdn, "wv": wv, "mm": mm,
            "xsh": xsh, "wsgu": wsgu, "wsd": wsd,
        })
    return C, ids, pos, in_maps


def _combine(C, ids, pos, results):
    # ---- combine: gather-sum of weighted routed rows + shared partials ----
    y_all = np.stack([r["yrt"].astype(np.float32) for r in results])  # [8, EPC, C, H]
    y_flat = y_all.reshape(N_EXPERTS * C, HIDDEN)
    G = ids * C + pos                                       # [T, 6]
    routed = y_flat[G].sum(axis=1)
    shared = np.sum([r["ysh"].astype(np.float32) for r in results], axis=0)
    out = routed + shared
    return out.reshape(1, T, HIDDEN).astype(np.float32)


def kernel(hidden_states, gate_w, w_gate_up, w_down, ws_gate_up, ws_down):
    global LAST_RESULTS
    C, ids, pos, in_maps = _prepare(
        hidden_states, gate_w, w_gate_up, w_down, ws_gate_up, ws_down
    )

    if C not in _PROGRAM_CACHE:
        _PROGRAM_CACHE[C] = _build_program(C)
    nc = _PROGRAM_CACHE[C]

    from concourse.bass_utils import run_bass_kernel_spmd
    res = run_bass_kernel_spmd(
        nc, in_maps, list(range(N_CORES)),
        trace=bool(os.environ.get("MOE_KERNEL_TRACE")),
    )
    LAST_RESULTS = res
    return _combine(C, ids, pos, res.results)
